# revision 17
# baseline (speedup 1.0000x reference)
"""VAE (4x LSTM-512 + reparameterize + dense) on 8 trn2 cores, data-parallel.

Per core (B_local=64, batch in partitions 0..63):
  - Recurrent matmuls use concat-K: gates = [inp, h] @ [[Wx],[Wh]], PSUM-
    accumulated over K-tiles of 128; weights SBUF-resident as float32r
    (full-rate fp32 on the trn2 PE, ~2.6e-4 rel err end-to-end).
  - Two skewed scan loops of T+1 ticks: encoder tick t runs e0 step t and
    e1 step t-1 plus the reparameterization (zm/zv dense, exp, z) fused
    in-tick; decoder tick runs d0/d1 the same way plus the output dense.
  - h [64,512] is transposed back to [512,64] k-tiles on the PE each tick
    (transpose-mode matmul via identity) for the next step's lhsT; the
    z.T sequence stays SBUF-resident between the two loops.
  - x is pre-transposed on the host to [D,T,B] so x_t DMAs straight in as
    a lhsT k-tile; when all bias vectors are zero (as in this problem's
    setup_inputs) the bias-inject matmuls are compiled out.
  - Gate biases (when nonzero) are injected with a K=1 ones-row matmul.
"""
import numpy as np

import concourse.bass as bass
import concourse.bacc as bacc
import concourse.mybir as mybir
import concourse.tile as tile
from concourse.bass_utils import run_bass_kernel_spmd

B, T, D, Z, U = 512, 128, 128, 64, 512
G = 4 * U  # 2048 gate width
NCORES = 8
BL = B // NCORES  # 64
AF = mybir.ActivationFunctionType
F32 = mybir.dt.float32
F32R = mybir.dt.float32r

_CACHED = {}


def _mm(nc, out, lhsT, rhs, start, stop=True, tile_position=None):
    nc.tensor.matmul(
        out, lhsT, rhs, start=start, stop=stop, tile_position=tile_position,
    )


def build_nc(t_steps=T, with_bias=True):
    nc = bacc.Bacc(name="vae_lstm", num_devices=NCORES)

    xT_d = nc.dram_tensor("xT", [D, t_steps, BL], F32R, kind="ExternalInput")
    eps = nc.dram_tensor("epsilon", [BL, t_steps, Z], F32, kind="ExternalInput")
    win = {}
    for L, din in (("e0", D), ("e1", U), ("d0", Z), ("d1", U)):
        win[L + "_Wx"] = nc.dram_tensor(L + "_Wx", [din, G], F32R, kind="ExternalInput")
        win[L + "_Wh"] = nc.dram_tensor(L + "_Wh", [U, G], F32R, kind="ExternalInput")
        win[L + "_b"] = nc.dram_tensor(L + "_b", [G], F32R, kind="ExternalInput")
    Wm = nc.dram_tensor("Wm", [U, Z], F32R, kind="ExternalInput")
    bm = nc.dram_tensor("bm", [Z], F32R, kind="ExternalInput")
    Wv = nc.dram_tensor("Wv", [U, Z], F32R, kind="ExternalInput")
    bv = nc.dram_tensor("bv", [Z], F32R, kind="ExternalInput")
    Wo = nc.dram_tensor("Wo", [U, D], F32R, kind="ExternalInput")
    bo = nc.dram_tensor("bo", [D], F32R, kind="ExternalInput")
    ones1 = nc.dram_tensor("ones1", [1, 128], F32R, kind="ExternalInput")
    ident = nc.dram_tensor("ident", [128, 64], F32, kind="ExternalInput")
    zpad = nc.dram_tensor("zpad", [128, 2048], F32R, kind="ExternalInput")

    rec = nc.dram_tensor("rec", [BL, t_steps, D], F32, kind="ExternalOutput")
    o_ezv = nc.dram_tensor("o_ezv", [BL, t_steps, Z], F32, kind="ExternalOutput")
    o_zm = nc.dram_tensor("o_zm", [BL, t_steps, Z], F32, kind="ExternalOutput")
    o_zv = nc.dram_tensor("o_zv", [BL, t_steps, Z], F32, kind="ExternalOutput")
    o_z = nc.dram_tensor("o_z", [BL, t_steps, Z], F32, kind="ExternalOutput")

    with tile.TileContext(nc) as tc:
        with (
            tc.tile_pool(name="persist", bufs=1) as pp,
            tc.tile_pool(name="wbig", bufs=1) as wp,
            tc.tile_pool(name="xin", bufs=3) as xp,
            tc.tile_pool(name="sig", bufs=2) as sp,
            tc.tile_pool(name="hT", bufs=3) as hp,
            tc.tile_pool(name="outs", bufs=3) as op,
            tc.tile_pool(name="gates", bufs=5, space="PSUM") as gp,
            tc.tile_pool(name="ps_small", bufs=1, space="PSUM") as qp,
            tc.tile_pool(name="ps_tp", bufs=2, space="PSUM") as tp,
        ):
            # ---- constants ----
            ones = pp.tile([1, 128], F32R, name="ones")
            nc.sync.dma_start(out=ones, in_=ones1[:, :])
            I64 = pp.tile([128, 64], F32, name="I64")
            nc.sync.dma_start(out=I64, in_=ident[:, :])

            def load_wcat(name, wx, wh, din):
                nk = din // 128 if din >= 128 else 1
                w = wp.tile([128, nk + 4, G], F32R, name="wA" if nk == 1 else "wB")
                if din < 128:
                    # zero-pad unused K rows of the input k-tile
                    nc.sync.dma_start(out=w[din:128, 0, :], in_=zpad[din:128, :])
                    nc.sync.dma_start(out=w[:din, 0, :], in_=wx[:, :])
                else:
                    for j in range(nk):
                        nc.sync.dma_start(
                            out=w[:, j, :], in_=wx[j * 128 : (j + 1) * 128, :]
                        )
                for j in range(4):
                    nc.sync.dma_start(
                        out=w[:, nk + j, :], in_=wh[j * 128 : (j + 1) * 128, :]
                    )
                return w, nk

            def load_bias(tag, bsrc):
                bt = pp.tile([1, G], F32R, name=tag)
                nc.sync.dma_start(out=bt, in_=bsrc[None, :])
                return bt

            # encoder weights
            w_e0, nk_e0 = load_wcat("w_e0", win["e0_Wx"], win["e0_Wh"], D)
            w_e1, nk_e1 = load_wcat("w_e1", win["e1_Wx"], win["e1_Wh"], U)
            b_e0 = load_bias("b_A", win["e0_b"])
            b_e1 = load_bias("b_B", win["e1_b"])
            wmv = pp.tile([128, 4, 2 * Z], F32R, name="wmv")
            for j in range(4):
                nc.sync.dma_start(out=wmv[:, j, 0:Z], in_=Wm[j * 128 : (j + 1) * 128, :])
                nc.sync.dma_start(out=wmv[:, j, Z : 2 * Z], in_=Wv[j * 128 : (j + 1) * 128, :])
            bmv = pp.tile([1, 2 * Z], F32R, name="bmv")
            nc.sync.dma_start(out=bmv[:, 0:Z], in_=bm[None, :])
            nc.sync.dma_start(out=bmv[:, Z : 2 * Z], in_=bv[None, :])

            # persistent state
            c_ea = pp.tile([64, U], F32, name="c_A")
            c_eb = pp.tile([64, U], F32, name="c_B")
            nc.vector.memset(c_ea, 0.0)
            nc.vector.memset(c_eb, 0.0)
            zT_seq = pp.tile([64, t_steps * 64], F32R, name="zT_seq")

            hT_prev = {}
            for nm in ("h0", "h1"):
                t0 = hp.tile([128, 4 * 64], F32R, name="haT" if nm in ("h0", "g0") else "hbT")
                nc.sync.dma_start(out=t0, in_=zpad[:, 0 : 4 * 64])
                hT_prev[nm] = t0

            tc.strict_bb_all_engine_barrier()

            def lstm_tick(
                do_a, do_b, w_a, nk_a, w_b, b_a, b_b, inT_a, haT, hbT, c_a, c_b
            ):
                """One tick: layer A step t, layer B step t-1 (split tiles,
                everything on partitions 0..63 except hT k-tiles)."""
                acts = {}
                gts_all = {}
                for lyr, do, bia in (("a", do_a, b_a), ("b", do_b, b_b)):
                    if not do:
                        continue
                    gts = []
                    for n in range(4):
                        g = gp.tile([64, 512], F32, name="g")
                        cs = slice(n * 512, (n + 1) * 512)
                        _mm(nc, g[:, :], ones[:, 0:64], bia[:, cs], start=True)
                        if lyr == "a":
                            for j, kt in enumerate(inT_a):
                                _mm(nc, g[:, :], kt, w_a[: kt.shape[0], j, cs], start=False)
                            for j in range(4):
                                _mm(nc, g[:, :], haT[:, j * 64 : (j + 1) * 64],
                                    w_a[:, nk_a + j, cs], start=False)
                        else:
                            for j in range(4):
                                _mm(nc, g[:, :], haT[:, j * 64 : (j + 1) * 64],
                                    w_b[:, j, cs], start=False)
                            for j in range(4):
                                _mm(nc, g[:, :], hbT[:, j * 64 : (j + 1) * 64],
                                    w_b[:, 4 + j, cs], start=False)
                        gts.append(g)
                    gts_all[lyr] = gts
                for lyr, do in (("a", do_a), ("b", do_b)):
                    if not do:
                        continue
                    gts = gts_all[lyr]
                    c_st = c_a if lyr == "a" else c_b
                    si = sp.tile([64, U], F32, name="si" + lyr)
                    sf = sp.tile([64, U], F32, name="sf" + lyr)
                    tg = sp.tile([64, U], F32, name="tg" + lyr)
                    so = sp.tile([64, U], F32, name="so" + lyr)
                    nc.scalar.activation(si[:, :], gts[0][:, :], AF.Sigmoid)
                    nc.scalar.activation(sf[:, :], gts[1][:, :], AF.Sigmoid)
                    nc.scalar.activation(tg[:, :], gts[2][:, :], AF.Tanh)
                    nc.scalar.activation(so[:, :], gts[3][:, :], AF.Sigmoid)
                    nc.vector.tensor_mul(sf[:, :], sf[:, :], c_st[:, :])
                    nc.vector.tensor_mul(si[:, :], si[:, :], tg[:, :])
                    nc.vector.tensor_add(c_st[:, :], sf[:, :], si[:, :])
                    nc.scalar.activation(tg[:, :], c_st[:, :], AF.Tanh)
                    nc.vector.tensor_mul(so[:, :], so[:, :], tg[:, :])
                    acts[lyr] = so  # = h

                haT_new, hbT_new = haT, hbT
                if do_a:
                    haT_new = hp.tile([128, 4 * 64], F32R, name="haT")
                    tps = tp.tile([128, 4 * 64], F32, name="tps")
                    for j in range(4):
                        nc.tensor.transpose(
                            tps[:, j * 64 : (j + 1) * 64],
                            acts["a"][:, j * 128 : (j + 1) * 128],
                            I64[0:64, :],
                        )
                    nc.vector.tensor_copy(haT_new[:, :], tps[:, :])
                if do_b:
                    hbT_new = hp.tile([128, 4 * 64], F32R, name="hbT")
                    tps = tp.tile([128, 4 * 64], F32, name="tps")
                    for j in range(4):
                        nc.tensor.transpose(
                            tps[:, j * 64 : (j + 1) * 64],
                            acts["b"][:, j * 128 : (j + 1) * 128],
                            I64[0:64, :],
                        )
                    nc.vector.tensor_copy(hbT_new[:, :], tps[:, :])
                return haT_new, hbT_new

            # ================= encoder =================
            for t in range(t_steps + 1):
                do_a = t < t_steps
                do_b = t >= 1
                inT_a = []
                if do_a:
                    xT = xp.tile([128, 64], F32R, name="xT")
                    nc.sync.dma_start(out=xT, in_=xT_d[:, t, :])
                    inT_a = [xT[:, :]]
                h0T_new, h1T_new = lstm_tick(
                    do_a, do_b, w_e0, nk_e0, w_e1, b_e0, b_e1,
                    inT_a, hT_prev["h0"], hT_prev["h1"], c_ea, c_eb,
                )
                if do_b:
                    s = t - 1  # encoder step index for z outputs
                    # zm|zv = h1 @ [Wm|Wv] + [bm|bv]
                    zp = qp.tile([64, 128], F32, name="small_ps")
                    if with_bias:
                        _mm(nc, zp[:, :], ones[:, 0:64], bmv[:, :], start=True)
                    for j in range(4):
                        _mm(
                            nc, zp[:, :],
                            h1T_new[:, j * 64 : (j + 1) * 64],
                            wmv[:, j, :], start=(not with_bias and j == 0),
                        )
                    zm = op.tile([64, Z], F32, name="zm")
                    zv = op.tile([64, Z], F32, name="zv")
                    sde = op.tile([64, Z], F32, name="sde")  # exp(0.5 zv)
                    ezv = op.tile([64, Z], F32, name="ezv")
                    zt = op.tile([64, Z], F32, name="zt")
                    ep = xp.tile([64, Z], F32, name="ep")
                    nc.sync.dma_start(out=ep, in_=eps[:, s, :])
                    nc.scalar.copy(zm[:, :], zp[:, 0:Z])
                    nc.scalar.copy(zv[:, :], zp[:, Z : 2 * Z])
                    nc.scalar.activation(sde[:, :], zp[:, Z : 2 * Z], AF.Exp, scale=0.5)
                    nc.vector.tensor_mul(ezv[:, :], sde[:, :], sde[:, :])
                    nc.vector.tensor_mul(zt[:, :], sde[:, :], ep[:, :])
                    nc.vector.tensor_add(zt[:, :], zt[:, :], zm[:, :])
                    nc.sync.dma_start(out=o_zm[:, s, :], in_=zm[:, :])
                    nc.sync.dma_start(out=o_zv[:, s, :], in_=zv[:, :])
                    nc.sync.dma_start(out=o_ezv[:, s, :], in_=ezv[:, :])
                    nc.sync.dma_start(out=o_z[:, s, :], in_=zt[:, :])
                    ztp = tp.tile([128, 4 * 64], F32, name="tps")
                    nc.tensor.transpose(ztp[0:64, 0:64], zt[:, :], I64[0:64, :])
                    nc.vector.tensor_copy(zT_seq[:, s * 64 : (s + 1) * 64], ztp[0:64, 0:64])
                hT_prev["h0"], hT_prev["h1"] = h0T_new, h1T_new

            # ================= decoder =================
            w_d0, nk_d0 = load_wcat("w_d0", win["d0_Wx"], win["d0_Wh"], Z)
            w_d1, nk_d1 = load_wcat("w_d1", win["d1_Wx"], win["d1_Wh"], U)
            b_d0 = load_bias("b_A", win["d0_b"])
            b_d1 = load_bias("b_B", win["d1_b"])
            wo = pp.tile([128, 4, D], F32R, name="wo")
            for j in range(4):
                nc.sync.dma_start(out=wo[:, j, :], in_=Wo[j * 128 : (j + 1) * 128, :])
            bo_t = pp.tile([1, D], F32R, name="bo_t")
            nc.sync.dma_start(out=bo_t, in_=bo[None, :])
            c_da = pp.tile([64, U], F32, name="c_A")
            c_db = pp.tile([64, U], F32, name="c_B")
            nc.vector.memset(c_da, 0.0)
            nc.vector.memset(c_db, 0.0)
            for nm in ("g0", "g1"):
                t0 = hp.tile([128, 4 * 64], F32R, name="haT" if nm in ("h0", "g0") else "hbT")
                nc.sync.dma_start(out=t0, in_=zpad[:, 0 : 4 * 64])
                hT_prev[nm] = t0
            tc.strict_bb_all_engine_barrier()

            for t in range(t_steps + 1):
                do_a = t < t_steps
                do_b = t >= 1
                inT_a = []
                if do_a:
                    inT_a = [zT_seq[:, t * 64 : (t + 1) * 64]]
                g0T_new, g1T_new = lstm_tick(
                    do_a, do_b, w_d0, nk_d0, w_d1, b_d0, b_d1,
                    inT_a, hT_prev["g0"], hT_prev["g1"], c_da, c_db,
                )
                if do_b:
                    s = t - 1
                    rp = qp.tile([64, 128], F32, name="small_ps")
                    if with_bias:
                        _mm(nc, rp[:, :], ones[:, 0:64], bo_t[:, :], start=True)
                    for j in range(4):
                        _mm(
                            nc, rp[:, :],
                            g1T_new[:, j * 64 : (j + 1) * 64],
                            wo[:, j, :], start=(not with_bias and j == 0),
                        )
                    ro = op.tile([64, D], F32, name="ro")
                    nc.scalar.copy(ro[:, :], rp[:, :])
                    nc.sync.dma_start(out=rec[:, s, :], in_=ro[:, :])
                hT_prev["g0"], hT_prev["g1"] = g0T_new, g1T_new

    nc.compile()
    return nc


def kernel(**inputs):
    t_steps = inputs["x"].shape[1]
    wb = any(
        np.any(np.asarray(inputs[k]))
        for k in ("e0_b", "e1_b", "d0_b", "d1_b", "bm", "bv", "bo")
    )
    key = ("nc", t_steps, wb)
    if key not in _CACHED:
        _CACHED[key] = build_nc(t_steps, with_bias=wb)
    nc = _CACHED[key]

    ones1 = np.ones((1, 128), np.float32)
    ident = np.vstack([np.eye(64, dtype=np.float32)] * 2)

    shared = {k: np.ascontiguousarray(np.asarray(v, np.float32))
              for k, v in inputs.items() if k not in ("x", "epsilon")}
    shared["ones1"] = ones1
    shared["ident"] = ident
    shared["zpad"] = np.zeros((128, 2048), np.float32)

    in_maps = []
    for c in range(NCORES):
        m = dict(shared)
        m["xT"] = np.ascontiguousarray(
            inputs["x"][c * BL : (c + 1) * BL].transpose(2, 1, 0).astype(np.float32))
        m["epsilon"] = np.ascontiguousarray(inputs["epsilon"][c * BL : (c + 1) * BL])
        in_maps.append(m)

    rs = _run_cached(nc, key, in_maps)
    cat = lambda nm: np.concatenate([rs[c][nm] for c in range(NCORES)], axis=0)
    return (cat("rec"), cat("o_ezv"), cat("o_zm"), cat("o_zv"), cat("o_z"))


def _run_cached(nc, key, in_maps):
    """Like bass2jax.run_bass_via_pjrt but with the jitted executable cached
    across kernel() calls (fresh jit closures defeat jax's cache otherwise)."""
    try:
        import jax
        from jax.sharding import Mesh, PartitionSpec
        from jax.experimental.shard_map import shard_map
        from concourse import bass2jax as b2j
        import concourse.mybir as mybir

        b2j.install_neuronx_cc_hook()
        ck = ("exec",) + key
        if ck not in _CACHED:
            in_names, out_names, out_avals, zero_outs = [], [], [], []
            pname = nc.partition_id_tensor.name if nc.partition_id_tensor else None
            for alloc in nc.m.functions[0].allocations:
                if not isinstance(alloc, mybir.MemoryLocationSet):
                    continue
                name = alloc.memorylocations[0].name
                if alloc.kind == "ExternalInput":
                    if name != pname:
                        in_names.append(name)
                elif alloc.kind == "ExternalOutput":
                    out_names.append(name)
                    shape = tuple(alloc.tensor_shape)
                    dtype = mybir.dt.np(alloc.dtype)
                    out_avals.append(jax.core.ShapedArray(shape, dtype))
                    zero_outs.append(np.zeros(shape, dtype))
            n_params = len(in_names)
            all_names = in_names + out_names + ([pname] if pname else [])

            def _body(*args):
                operands = list(args)
                if pname is not None:
                    operands.append(b2j.partition_id_tensor())
                return tuple(
                    b2j._bass_exec_p.bind(
                        *operands,
                        out_avals=tuple(out_avals),
                        in_names=tuple(all_names),
                        out_names=tuple(out_names),
                        lowering_input_output_aliases=(),
                        sim_require_finite=True,
                        sim_require_nnan=True,
                        nc=nc,
                    )
                )

            devices = jax.devices()[:NCORES]
            mesh = Mesh(np.asarray(devices), ("core",))
            nio = n_params + len(out_names)
            sharded = jax.jit(
                shard_map(
                    _body, mesh=mesh, in_specs=(PartitionSpec("core"),) * nio,
                    out_specs=(PartitionSpec("core"),) * len(out_names),
                    check_rep=False,
                ),
                donate_argnums=tuple(range(n_params, nio)),
                keep_unused=True,
            )
            _CACHED[ck] = (sharded, in_names, out_names, out_avals, zero_outs)
        sharded, in_names, out_names, out_avals, zero_outs = _CACHED[ck]
        concat_in = [
            np.concatenate([np.asarray(m[n]) for m in in_maps], axis=0)
            for n in in_names
        ]
        concat_zeros = [
            np.zeros((NCORES * z.shape[0], *z.shape[1:]), z.dtype) for z in zero_outs
        ]
        out_arrs = sharded(*concat_in, *concat_zeros)
        return [
            {
                n: np.asarray(out_arrs[i]).reshape(NCORES, *out_avals[i].shape)[c]
                for i, n in enumerate(out_names)
            }
            for c in range(NCORES)
        ]
    except Exception:
        return run_bass_kernel_spmd(nc, in_maps, core_ids=list(range(NCORES))).results
